# revision 38
# baseline (speedup 1.0000x reference)
"""Data2VecVision self-attention Bass kernel for 8 Trainium2 NeuronCores.

Sharding: data-parallel over batch (64 = 8 cores x 8 batches/core).
Measured (NTFF profile, core 0): ~136-138 us/core, rel err 1.83e-2
(vs fp32 ref; the error is dominated by the deterministic fp8 QK
quantization and is stable run-to-run).

v3: software-pipelined single-stream emission + fp8 QK projections.
  - Attention math as v1 (exp(s+b)=exp(s)*exp(b) with host-baked
    exp(bias) table, V padded per-head with a ones column so softmax
    sums fall out of the context matmul for free).
  - Q/K projections run as fp8(e4m3) DoubleRow matmuls: two 128-row
    contraction chunks per pass -> half the projection matmul time.
    Wq is host-scaled by 64/8, Wk by 64 (lifts the 0.02-scale weights
    out of the fp8 subnormal range); the resulting 4096x score scale
    is divided out for free by the Exp activation's scale operand.
  - The QK projection of batch-pair p+1 and the V projection (fp16) of
    pair p+1 are emitted as PE gap fillers inside the attention stream
    of pair p, so Scalar/Vector elementwise work overlaps projection
    matmuls across the whole kernel, not just in an attention phase.
  - Scores for a head pair go into one 2-bank PSUM tile [128,1024]
    (head h at columns h*512): one exp and one exp(bias) multiply per
    head PAIR (halves ACT/DVE op-count overhead). Head-alternating
    scores matmuls let LDWEIGHTS pull ahead across row groups.
  - First batch pair emits all fronts before any ctx matmul so the
    late-arriving V/wv DMAs never block the PE pipeline; the last pair
    switches to per-head score tiles rotating through the idle wp pool
    (double-buffers the exp chain) with eb-multiplies split DVE/GpSimd.
  - 10+3 dummy warm-up matmuls on garbage SBUF run during the
    input-DMA window so the PE HAM clock-gate is released before real
    matmuls start; a dummy exp hoists the ACT table load into the
    preamble. Inputs stream as large transfers ordered by first use
    over three DMA queues.
  - PSUM budget (8 banks): wp 2 (QK/V groups + tail score tiles),
    sp 2 (paired-score tile), cp 4 (ctx accumulators + warm-up sink).
"""

import ml_dtypes
import numpy as np

import concourse.bacc as bacc
import concourse.mybir as mybir
import concourse.tile as tile
from concourse.bass_utils import run_bass_kernel_spmd

F32 = mybir.dt.float32
F16 = mybir.dt.float16
F8 = mybir.dt.float8e4
NPF8 = ml_dtypes.float8_e4m3
AF = mybir.ActivationFunctionType
ALU = mybir.AluOpType
DR = mybir.MatmulPerfMode.DoubleRow

# fp8 (e4m3) QK projections via DoubleRow matmuls: Wq is scaled by 64/8,
# Wk by 64 (lifts the 0.02-scale weights out of the fp8 subnormal range);
# the resulting 4096x score scale is divided back out for free by the
# `scale` operand of the Exp activation. Measured rel err ~1.8e-2 for
# both-sides fp8, ~1.3e-2 for K-only (gate 2e-2).
FP8_Q = True
FP8_K = True
QSCALE = (8.0 if FP8_Q else 1.0 / 8.0)   # qt = QSCALE * (Wq hs + bq)
KSCALE = (64.0 if FP8_K else 1.0)        # kt = KSCALE * (Wk hs)
EXP_SCALE = 1.0 / (QSCALE * KSCALE * 8.0)  # scores_true = psum * EXP_SCALE

N_CORES = 8
B = 64
NB = B // N_CORES          # batches per core
S = 197
HID = 768
HEADS = 12
D = 64
NHP = HEADS // 2           # head pairs
NCH = HID // 128           # 6 contraction chunks
NST = 4                    # projection s-tiles per core (one per batch pair)
SW = NB * S // NST         # 394, projection moving width
CORE_S = NB * S            # 1576
JC = [(0, 128), (128, 69)]   # j/i chunk (offset, len)


def _relative_position_index(h, w):
    coords = np.stack(np.meshgrid(np.arange(h), np.arange(w), indexing="ij")).reshape(2, -1)
    rel = coords[:, :, None] - coords[:, None, :]
    rel = rel.transpose(1, 2, 0).astype(np.int64)
    rel[:, :, 0] += h - 1
    rel[:, :, 1] += w - 1
    rel[:, :, 0] *= 2 * w - 1
    area = h * w
    nrd = (2 * h - 1) * (2 * w - 1) + 3
    idx = np.zeros((area + 1, area + 1), dtype=np.int64)
    idx[1:, 1:] = rel.sum(-1)
    idx[0, :] = nrd - 3
    idx[:, 0] = nrd - 2
    idx[0, 0] = nrd - 1
    return idx


def build_nc(reps=1):
    nc = bacc.Bacc("TRN2", target_bir_lowering=False, debug=False)

    hsT_d = nc.dram_tensor("hsT", [NST, NCH, 128, SW], F16, kind="ExternalInput").ap()
    QKT = F8 if FP8_Q else F16
    hs8_d = (nc.dram_tensor("hs8T", [NST, NCH, 128, SW], F8, kind="ExternalInput").ap()
             if (FP8_Q or FP8_K) else None)
    wq_d = nc.dram_tensor("wqT", [NCH, 128, HID], QKT, kind="ExternalInput").ap()  # c-major
    wk_d = nc.dram_tensor("wkT", [NCH, 128, HID], F8 if FP8_K else F16,
                          kind="ExternalInput").ap()  # c-major
    wv_d = nc.dram_tensor("wvT", [NCH, 128, HID], F16, kind="ExternalInput").ap()
    bq_d = nc.dram_tensor("bqc", [NCH, 128, 1], F32, kind="ExternalInput").ap()
    bv_d = nc.dram_tensor("bvb", [128, HID], F32, kind="ExternalInput").ap()
    eb_d = nc.dram_tensor("expb", [NHP, 128, 788], F16, kind="ExternalInput").ap()
    y_d = nc.dram_tensor("y", [NB, S, HID], F32, kind="ExternalOutput").ap()

    with tile.TileContext(nc) as tc:
        with (
            tc.tile_pool(name="res", bufs=1) as res,
            tc.tile_pool(name="vpad", bufs=NB * 2) as vpad_pool,
            tc.tile_pool(name="er", bufs=6) as er_pool,
            tc.tile_pool(name="et", bufs=6) as et_pool,
            tc.tile_pool(name="rt", bufs=8) as rt_pool,
            tc.tile_pool(name="ot", bufs=4) as ot_pool,
            tc.tile_pool(name="wp", bufs=2, space="PSUM") as wps,
            tc.tile_pool(name="sp", bufs=1, space="PSUM") as sps,
            tc.tile_pool(name="cp", bufs=4, space="PSUM") as cps_pool,
        ):
            hs_sb = res.tile([128, NCH * CORE_S], F16)
            hs8_sb = (res.tile([128, NCH * CORE_S], F8, name="hs8_sb")
                      if (FP8_Q or FP8_K) else None)
            wq_sb = res.tile([128, NCH * HID], QKT)
            wk_sb = res.tile([128, NCH * HID], F8 if FP8_K else F16)
            wv_sb = res.tile([128, NCH * HID], F16)
            bq_sb = res.tile([128, NCH], F32)
            bv_sb = res.tile([128, HID], F32)
            eb_sb = res.tile([128, NHP * 788], F16)
            qt_sb = res.tile([128, NCH * CORE_S], F16)
            kt_sb = res.tile([128, NCH * CORE_S + 64], F16)
            scratch = res.tile([128, 518], F16)   # garbage; warm-up only
            dact = res.tile([128, 2], F16)
            nc.vector.memset(kt_sb[:, NCH * CORE_S:], 0.0)
            vpad = [[vpad_pool.tile([128, HEADS * 65], F16, tag="vp",
                                    name=f"vpad_{b}_{j}") for j in range(2)]
                    for b in range(NB)]

            hs_c = hs_sb.rearrange("p (c x) -> p c x", c=NCH)
            hs8_c = (hs8_sb.rearrange("p (c x) -> p c x", c=NCH)
                     if hs8_sb is not None else None)
            wq_c = wq_sb.rearrange("p (c x) -> p c x", c=NCH)
            wk_c = wk_sb.rearrange("p (c x) -> p c x", c=NCH)
            wq_4 = wq_sb.rearrange("p (c h m) -> p c h m", c=NCH, h=NCH)
            wk_4 = wk_sb.rearrange("p (c h m) -> p c h m", c=NCH, h=NCH)
            wv_c = wv_sb.rearrange("p (c x) -> p c x", c=NCH)
            eb_c = eb_sb.rearrange("p (c x) -> p c x", c=NHP)

            for _ in range(reps):
                # ---- PE warm-up + ACT table-load hoist (zero inputs,
                # outputs never read; runs during the input-DMA window and
                # releases the HAM clock-gate before real matmuls start).
                nc.vector.memset(scratch[:], 0.0)
                nc.scalar.activation(dact[:], scratch[:, :2], AF.Exp)
                dummy_ps = cps_pool.tile([128, 390], F32, tag="c", name="dummy")
                for _k in range(10):
                    nc.tensor.matmul(dummy_ps[:], scratch[:, :128],
                                     scratch[:, 128:518], start=True, stop=True)

                # ---- input DMAs, ordered by first use, rotated over 3 queues ----
                def hs_st(st, c0, c1):
                    return (hs_c[:, c0:c1, st * SW:(st + 1) * SW],
                            hsT_d[st, c0:c1].rearrange("c p x -> p c x"))
                def hs8_st(st, c0, c1):
                    return (hs8_c[:, c0:c1, st * SW:(st + 1) * SW],
                            hs8_d[st, c0:c1].rearrange("c p x -> p c x"))
                fp8 = FP8_Q or FP8_K
                qk_src = hs8_st if fp8 else hs_st
                xfers = [
                    (wq_c[:, 0, :], wq_d[0]),
                    qk_src(0, 0, 3),
                    qk_src(0, 3, 6),
                    (bq_sb[:], bq_d[:, :, 0].rearrange("c p -> p c")),
                    (wk_c[:, 0, :], wk_d[0]),
                    (wq_c[:, 1, :], wq_d[1]),
                    (wk_c[:, 1, :], wk_d[1]),
                ]
                xfers += [
                    (wq_c[:, 2, :], wq_d[2]),
                    (wk_c[:, 2, :], wk_d[2]),
                    (eb_c[:, 0, :], eb_d[0]),
                    (wq_c[:, 3, :], wq_d[3]),
                    (wk_c[:, 3, :], wk_d[3]),
                    (wq_c[:, 4, :], wq_d[4]),
                    (wk_c[:, 4, :], wk_d[4]),
                    (wq_c[:, 5, :], wq_d[5]),
                    (wk_c[:, 5, :], wk_d[5]),
                    qk_src(1, 0, 3),
                    qk_src(1, 3, 6),
                    (wv_c[:, 0:2, :], wv_d[0:2].rearrange("c p x -> p c x")),
                    (wv_c[:, 2:4, :], wv_d[2:4].rearrange("c p x -> p c x")),
                    (wv_c[:, 4:6, :], wv_d[4:6].rearrange("c p x -> p c x")),
                ]
                if fp8:
                    xfers += [hs_st(0, 0, 3), hs_st(0, 3, 6)]
                xfers += [
                    (bv_sb[:], bv_d[:]),
                    (eb_c[:, 1, :], eb_d[1]),
                    (eb_c[:, 2, :], eb_d[2]),
                    qk_src(2, 0, 6),
                    (eb_c[:, 3:, :], eb_d[3:].rearrange("c p x -> p c x")),
                    qk_src(3, 0, 6),
                ]
                if fp8:
                    xfers += [hs_st(1, 0, 3), hs_st(1, 3, 6),
                              hs_st(2, 0, 3), hs_st(2, 3, 6),
                              hs_st(3, 0, 3), hs_st(3, 3, 6)]
                dma_engs = [nc.sync, nc.gpsimd, nc.scalar]
                for xi, (dst, src) in enumerate(xfers):
                    dma_engs[xi % 3].dma_start(dst, src)

                # ---- QK projection group: one (st, c, q|k) psum group ----
                def emit_qk(st, c, which, pool_tag):
                    pool, tag = pool_tag
                    pp = pool.tile([128, 1024] if tag == "s" else [128, SW], F32,
                                   tag=tag, name=f"{which}p_{st}_{c}")
                    use8 = FP8_Q if which == "q" else FP8_K
                    if use8:
                        w_4 = wq_4 if which == "q" else wk_4
                        for t in range(NCH // 2):
                            nc.tensor.matmul(
                                pp[:, :SW],
                                w_4[:, c, 2 * t:2 * t + 2, :],
                                hs8_c[:, 2 * t:2 * t + 2, st * SW:(st + 1) * SW],
                                start=(t == 0), stop=(t == NCH // 2 - 1),
                                perf_mode=DR)
                    else:
                        w_sb = wq_sb if which == "q" else wk_sb
                        for hch in range(NCH):
                            nc.tensor.matmul(
                                pp[:, :SW],
                                w_sb[:, c * HID + hch * 128: c * HID + (hch + 1) * 128],
                                hs_sb[:, hch * CORE_S + st * SW: hch * CORE_S + (st + 1) * SW],
                                start=(hch == 0), stop=(hch == NCH - 1))
                    if which == "q":
                        nc.vector.tensor_scalar_add(
                            qt_sb[:, c * CORE_S + st * SW: c * CORE_S + (st + 1) * SW],
                            pp[:, :SW], bq_sb[:, c:c + 1])
                    else:
                        nc.vector.tensor_copy(
                            kt_sb[:, c * CORE_S + st * SW: c * CORE_S + (st + 1) * SW],
                            pp[:, :SW])

                # ---- V projection for (batch, j-chunk) ----
                def emit_v(b, jci):
                    joff, jlen = JC[jci]
                    vt = vpad[b][jci]
                    ones_ap = vt[:jlen].rearrange("p (h c) -> p h c", h=HEADS)[:, :, 64:65]
                    nc.gpsimd.memset(ones_ap, 1.0)
                    scol = b * S + joff
                    for nt, (noff, nlen) in enumerate([(0, 512), (512, 256)]):
                        vp = wps.tile([128, 512], F32, tag="w",
                                      name=f"vp_{b}_{jci}_{nt}")
                        for c in range(NCH):
                            nc.tensor.matmul(
                                vp[:jlen, :nlen],
                                hs_sb[:, c * CORE_S + scol: c * CORE_S + scol + jlen],
                                wv_sb[:, c * HID + noff: c * HID + noff + nlen],
                                start=(c == 0), stop=(c == NCH - 1))
                        dst = vt[:jlen, nt * 8 * 65:].rearrange(
                            "p (h c) -> p h c", c=65)[:, :nlen // 64, :64]
                        nc.vector.tensor_tensor(
                            out=dst, in0=vp[:jlen, :nlen],
                            in1=bv_sb[:jlen, noff:noff + nlen],
                            op=ALU.add)

                # ---- prologue: QK(st0); V(b0)/V(b1) are the first fill
                # units of pair 0 so they land between attention fronts,
                # right when wv/hs arrive. Two extra warm-keeper dummies
                # bridge any DMA wait so the HAM gate stays open.
                pat = [(wps, "w"), (wps, "w"), (sps, "s")]
                gi = 0
                for st in (0, 1):
                    for c in range(NCH):
                        for which in ("q", "k"):
                            emit_qk(st, c, which, pat[gi % 3])
                            gi += 1

                # ---- attention ----
                def emit_front(b, hp, mul_eng):
                    col = hp * CORE_S + b * S
                    sp = sps.tile([128, 1024], F32, tag="s", name=f"sp_{b}_{hp}")
                    for jci in range(2):
                        for h in range(2):
                            nc.tensor.matmul(
                                sp[:, h * 512 + jci * S: h * 512 + jci * S + S],
                                kt_sb[h * 64:(h + 1) * 64,
                                      col + jci * 128: col + jci * 128 + 128],
                                qt_sb[h * 64:(h + 1) * 64, col: col + S],
                                start=True, stop=True)
                    er = er_pool.tile([128, 788], F16, tag="er", name=f"er_{b}_{hp}")
                    nc.scalar.activation(
                        er.rearrange("p (h x) -> p h x", h=2),
                        sp.rearrange("p (h x) -> p h x", h=2)[:, :, :394], AF.Exp,
                        scale=EXP_SCALE)
                    et = et_pool.tile([128, 788], F16, tag="et", name=f"et_{b}_{hp}")
                    mul_eng.tensor_tensor(
                        out=et[:], in0=er[:],
                        in1=eb_sb[:, hp * 788:(hp + 1) * 788], op=ALU.mult)
                    return [(et, 0), (et, 394)]

                def emit_front_single(b, hp):
                    # tail variant: per-head PSUM tiles from the (now idle)
                    # wp pool so the exp->scores chain double-buffers even
                    # though the sp pool has a single slot; eb multiplies
                    # alternate DVE/GpSimd to spread the un-overlapped tail.
                    col = hp * CORE_S + b * S
                    ets = []
                    for h in range(2):
                        sph = wps.tile([128, SW], F32, tag="w",
                                       name=f"sps_{b}_{hp}_{h}")
                        for jci in range(2):
                            nc.tensor.matmul(
                                sph[:, jci * S: jci * S + S],
                                kt_sb[h * 64:(h + 1) * 64,
                                      col + jci * 128: col + jci * 128 + 128],
                                qt_sb[h * 64:(h + 1) * 64, col: col + S],
                                start=True, stop=True)
                        er = er_pool.tile([128, 394], F16, tag="er",
                                          name=f"ers_{b}_{hp}_{h}")
                        nc.scalar.activation(er[:], sph[:, :SW], AF.Exp,
                                             scale=EXP_SCALE)
                        et = et_pool.tile([128, 394], F16, tag="et",
                                          name=f"ets_{b}_{hp}_{h}")
                        eng = nc.gpsimd if (hp + h) % 2 else nc.vector
                        eng.tensor_tensor(
                            out=et[:], in0=er[:],
                            in1=eb_sb[:, hp * 788 + h * 394: hp * 788 + (h + 1) * 394],
                            op=ALU.mult)
                        ets.append((et, 0))
                    return ets

                def emit_ctx(b, half, hpl, ets, cps):
                    for ici, (ioff, ilen) in enumerate(JC):
                        for h in range(2):
                            et, base = ets[h]
                            for jci, (joff, jlen) in enumerate(JC):
                                nc.tensor.matmul(
                                    cps[ici][:ilen, hpl * 130 + h * 65:
                                             hpl * 130 + (h + 1) * 65],
                                    et[:jlen, base + jci * S + ioff:
                                       base + jci * S + ioff + ilen],
                                    vpad[b][jci][:jlen,
                                                 ((half * 3 + hpl) * 2 + h) * 65:
                                                 ((half * 3 + hpl) * 2 + h + 1) * 65],
                                    start=(jci == 0), stop=(jci == 1))

                def attn(b, fill, single=False, fronts_first=False):
                    ot = [ot_pool.tile([128, HID], F32, tag="ot",
                                       name=f"ot_{b}_{i}") for i in range(2)]
                    for half in range(2):
                        cps = [cps_pool.tile([128, 390], F32, tag="c",
                                             name=f"cp_{b}_{half}_{i}")
                               for i in range(2)]
                        if fronts_first:
                            # first pair: V hasn't landed yet, so emit all
                            # fronts (which only need qt/kt/eb) before any
                            # ctx matmul touches vpad.
                            etss = []
                            for hpl in range(3):
                                etss.append(emit_front(b, half * 3 + hpl,
                                                       nc.vector))
                                fill()
                            fill()
                            for hpl in range(3):
                                emit_ctx(b, half, hpl, etss[hpl], cps)
                        else:
                            prev = None
                            for hpl in range(3):
                                hp = half * 3 + hpl
                                if single:
                                    ets = emit_front_single(b, hp)
                                else:
                                    ets = emit_front(b, hp, nc.vector)
                                fill()
                                if prev is not None:
                                    emit_ctx(b, half, prev[0], prev[1], cps)
                                prev = (hpl, ets)
                            emit_ctx(b, half, prev[0], prev[1], cps)
                            fill()
                        for ici, (ioff, ilen) in enumerate(JC):
                            r = rt_pool.tile([128, 6], F32, tag="rt",
                                             name=f"r_{b}_{half}_{ici}")
                            sums = cps[ici][:ilen].rearrange(
                                "p (g c) -> p g c", c=65)[:, :, 64:65]
                            nc.vector.reciprocal(r[:ilen], sums)
                            nc.vector.tensor_tensor(
                                out=ot[ici][:ilen, half * 384:(half + 1) * 384]
                                    .rearrange("p (g c) -> p g c", c=64),
                                in0=cps[ici][:ilen].rearrange(
                                    "p (g c) -> p g c", c=65)[:, :, :64],
                                in1=r[:ilen].broadcast_to([ilen, 6, 64]),
                                op=ALU.mult)
                    for ici, (ioff, ilen) in enumerate(JC):
                        nc.sync.dma_start(y_d[b, ioff:ioff + ilen, :],
                                          ot[ici][:ilen, :])

                for p in range(NST):
                    # QK of pair p+2 fills pair p's attention stream (pairs
                    # 0 and 1 were projected in the DMA-bound prologue);
                    # V of pair p+1 fills as before.
                    units = []
                    qk_units = ([(p + 2, c, w) for c in range(NCH)
                                 for w in ("q", "k")] if p + 2 < NST else [])
                    if p == 0:
                        # this pair's own V lands mid-stream, right before
                        # its first ctx matmuls need it
                        units += [("v", (0, 0)), ("v", (0, 1)),
                                  ("v", (1, 0)), ("v", (1, 1))]
                    if p + 1 < NST:
                        v_units = [(2 * p + 2, 0), (2 * p + 2, 1),
                                   (2 * p + 3, 0), (2 * p + 3, 1)]
                        qi = vi = 0
                        while qi < len(qk_units) or vi < len(v_units):
                            for _ in range(3):
                                if qi < len(qk_units):
                                    units.append(("qk", qk_units[qi])); qi += 1
                            if vi < len(v_units):
                                units.append(("v", v_units[vi])); vi += 1
                    units.reverse()

                    def fill():
                        if units:
                            kind, u = units.pop()
                            if kind == "qk":
                                st, c, w = u
                                emit_qk(st, c, w, (wps, "w"))
                            else:
                                emit_v(*u)

                    last = p == NST - 1
                    attn(2 * p, fill, single=last, fronts_first=(p == 0))
                    attn(2 * p + 1, fill, single=last, fronts_first=(p == 0))
                    while units:
                        fill()

    nc.compile()
    return nc


_NC_CACHE = {}


def _get_nc(reps=1):
    if reps not in _NC_CACHE:
        _NC_CACHE[reps] = build_nc(reps)
    return _NC_CACHE[reps]


def prep_inputs(hidden_states, Wq, bq, Wk, Wv, bv, bias_table):
    hidden_states = np.asarray(hidden_states, np.float32)
    Wq = np.asarray(Wq, np.float32)
    bq = np.asarray(bq, np.float32)
    Wk = np.asarray(Wk, np.float32)
    Wv = np.asarray(Wv, np.float32)
    bv = np.asarray(bv, np.float32)
    bias_table = np.asarray(bias_table, np.float32)

    def cmajor(wT):
        # [h_in, d_out] -> [c, p, hch*128+col] so one DMA covers one d_out chunk
        return np.ascontiguousarray(
            wT.reshape(NCH, 128, NCH, 128).transpose(2, 1, 0, 3).reshape(NCH, 128, HID))
    wqT = cmajor((Wq * QSCALE).T).astype(NPF8 if FP8_Q else np.float16)
    wkT = cmajor((Wk * KSCALE).T).astype(NPF8 if FP8_K else np.float16)
    wvT = np.ascontiguousarray(Wv.T).reshape(NCH, 128, HID).astype(np.float16)
    bqc = (bq * QSCALE).astype(np.float32).reshape(NCH, 128, 1)
    bvb = np.ascontiguousarray(np.broadcast_to(bv, (128, HID))).astype(np.float32)

    idx = _relative_position_index(14, 14)
    bias_full = bias_table[idx]              # [S, S, HEADS] (i, j, h)
    biasT = bias_full.transpose(2, 1, 0)     # [h, j, i]
    expb = np.zeros((NHP, 128, 788), np.float32)
    for c in range(NHP):
        for h in range(2):
            for jci, (joff, jlen) in enumerate(JC):
                expb[c, :jlen, h * 394 + jci * S: h * 394 + jci * S + S] = \
                    np.exp(biasT[2 * c + h, joff:joff + jlen, :])
    expb = expb.astype(np.float16)

    shared = {"wqT": wqT, "wkT": wkT, "wvT": wvT, "bqc": bqc, "bvb": bvb,
              "expb": expb}
    in_maps = []
    for core in range(N_CORES):
        hs_c = hidden_states[core * NB:(core + 1) * NB]      # [NB, S, HID]
        hsT = np.ascontiguousarray(hs_c.transpose(2, 0, 1).reshape(HID, CORE_S))
        hsT4 = np.ascontiguousarray(
            hsT.reshape(NCH, 128, NST, SW).transpose(2, 0, 1, 3))
        im = {"hsT": hsT4.astype(np.float16), **shared}
        if FP8_Q or FP8_K:
            im["hs8T"] = hsT4.astype(NPF8)
        in_maps.append(im)
    return in_maps


def run(in_maps, reps=1, **kw):
    nc = _get_nc(reps)
    res = run_bass_kernel_spmd(nc, in_maps, core_ids=list(range(N_CORES)), **kw)
    out = np.concatenate([res.results[c]["y"] for c in range(N_CORES)], axis=0)
    return out, res


def kernel(hidden_states, Wq, bq, Wk, Wv, bv, bias_table,
           resolution_h=224, resolution_w=224):
    assert int(resolution_h) == 224 and int(resolution_w) == 224, \
        "kernel compiled for 224x224 (window 14x14, S=197)"
    hidden_states = np.asarray(hidden_states)
    assert hidden_states.shape == (B, S, HID), hidden_states.shape
    in_maps = prep_inputs(hidden_states, Wq, bq, Wk, Wv, bv, bias_table)
    return run(in_maps, reps=1)[0]


# revision 40
# speedup vs baseline: 1.0618x; 1.0618x over previous
"""Data2VecVision self-attention Bass kernel for 8 Trainium2 NeuronCores.

Sharding: data-parallel over batch (64 = 8 cores x 8 batches/core).

v2: software-pipelined single-stream emission.
  - Same math as v1 (fp16 matmuls, exp(s+b)=exp(s)*exp(b) with host-baked
    exp(bias) table, V padded per-head with a ones column so softmax sums
    fall out of the context matmul, 1/8 folded into Wq/bq).
  - The QK projection of batch-pair p+1 and the V projection of pair p+1
    are emitted as PE gap fillers inside the attention stream of pair p,
    so Scalar/Vector elementwise work overlaps the projection matmuls
    across the whole kernel instead of only inside the attention phase.
  - Scores for a head pair go into one 2-bank PSUM tile [128,1024]
    (head h at columns h*512); one exp and one exp(bias) multiply per
    head PAIR instead of per head (halves ACT/DVE op-count overhead).
  - Scores matmuls alternate heads (row groups 0-63 / 64-127) so
    LDWEIGHTS of one head pulls ahead under the other head's matmul.
  - kt evacuation moved to Scalar (ACT Copy) to balance DVE.
  - 7 dummy warm-up matmuls on garbage SBUF run during the input-DMA
    window so the PE HAM clock-gate is at full rate when real matmuls
    start; a dummy exp hoists the ACT table load into the preamble.
  - Inputs DMAd as few large transfers ordered by first use.
"""

import ml_dtypes
import numpy as np

import concourse.bacc as bacc
import concourse.mybir as mybir
import concourse.tile as tile
from concourse.bass_utils import run_bass_kernel_spmd

F32 = mybir.dt.float32
F16 = mybir.dt.float16
F8 = mybir.dt.float8e4
NPF8 = ml_dtypes.float8_e4m3
AF = mybir.ActivationFunctionType
ALU = mybir.AluOpType
DR = mybir.MatmulPerfMode.DoubleRow

# fp8 (e4m3) QK projections via DoubleRow matmuls: Wq is scaled by 64/8,
# Wk by 64 (lifts the 0.02-scale weights out of the fp8 subnormal range);
# the resulting 4096x score scale is divided back out for free by the
# `scale` operand of the Exp activation. Measured rel err ~1.8e-2 for
# both-sides fp8, ~1.3e-2 for K-only (gate 2e-2).
FP8_Q = True
FP8_K = True
QSCALE = (8.0 if FP8_Q else 1.0 / 8.0)   # qt = QSCALE * (Wq hs + bq)
KSCALE = (64.0 if FP8_K else 1.0)        # kt = KSCALE * (Wk hs)
EXP_SCALE = 1.0 / (QSCALE * KSCALE * 8.0)  # scores_true = psum * EXP_SCALE

N_CORES = 8
B = 64
NB = B // N_CORES          # batches per core
S = 197
HID = 768
HEADS = 12
D = 64
NHP = HEADS // 2           # head pairs
NCH = HID // 128           # 6 contraction chunks
NST = 4                    # projection s-tiles per core (one per batch pair)
SW = NB * S // NST         # 394, projection moving width
CORE_S = NB * S            # 1576
JC = [(0, 128), (128, 69)]   # j/i chunk (offset, len)


def _relative_position_index(h, w):
    coords = np.stack(np.meshgrid(np.arange(h), np.arange(w), indexing="ij")).reshape(2, -1)
    rel = coords[:, :, None] - coords[:, None, :]
    rel = rel.transpose(1, 2, 0).astype(np.int64)
    rel[:, :, 0] += h - 1
    rel[:, :, 1] += w - 1
    rel[:, :, 0] *= 2 * w - 1
    area = h * w
    nrd = (2 * h - 1) * (2 * w - 1) + 3
    idx = np.zeros((area + 1, area + 1), dtype=np.int64)
    idx[1:, 1:] = rel.sum(-1)
    idx[0, :] = nrd - 3
    idx[:, 0] = nrd - 2
    idx[0, 0] = nrd - 1
    return idx


def build_nc(reps=1):
    nc = bacc.Bacc("TRN2", target_bir_lowering=False, debug=False)

    hsT_d = nc.dram_tensor("hsT", [NST, NCH, 128, SW], F16, kind="ExternalInput").ap()
    QKT = F8 if FP8_Q else F16
    hs8_d = (nc.dram_tensor("hs8T", [NST, NCH, 128, SW], F8, kind="ExternalInput").ap()
             if (FP8_Q or FP8_K) else None)
    wq_d = nc.dram_tensor("wqT", [NCH, 128, HID], QKT, kind="ExternalInput").ap()  # c-major
    wk_d = nc.dram_tensor("wkT", [NCH, 128, HID], F8 if FP8_K else F16,
                          kind="ExternalInput").ap()  # c-major
    wv_d = nc.dram_tensor("wvT", [NCH, 128, HID], F16, kind="ExternalInput").ap()
    bq_d = nc.dram_tensor("bqc", [NCH, 128, 1], F32, kind="ExternalInput").ap()
    bv_d = nc.dram_tensor("bvb", [128, HID], F32, kind="ExternalInput").ap()
    eb_d = nc.dram_tensor("expb", [NHP, 128, 788], F16, kind="ExternalInput").ap()
    y_d = nc.dram_tensor("y", [NB, S, HID], F32, kind="ExternalOutput").ap()

    with tile.TileContext(nc) as tc:
        with (
            tc.tile_pool(name="res", bufs=1) as res,
            tc.tile_pool(name="vpad", bufs=NB * 2) as vpad_pool,
            tc.tile_pool(name="er", bufs=6) as er_pool,
            tc.tile_pool(name="et", bufs=6) as et_pool,
            tc.tile_pool(name="rt", bufs=8) as rt_pool,
            tc.tile_pool(name="ot", bufs=4) as ot_pool,
            tc.tile_pool(name="wp", bufs=2, space="PSUM") as wps,
            tc.tile_pool(name="sp", bufs=1, space="PSUM") as sps,
            tc.tile_pool(name="cp", bufs=4, space="PSUM") as cps_pool,
        ):
            hs_sb = res.tile([128, NCH * CORE_S], F16)
            hs8_sb = (res.tile([128, NCH * CORE_S], F8, name="hs8_sb")
                      if (FP8_Q or FP8_K) else None)
            wq_sb = res.tile([128, NCH * HID], QKT)
            wk_sb = res.tile([128, NCH * HID], F8 if FP8_K else F16)
            wv_sb = res.tile([128, NCH * HID], F16)
            bq_sb = res.tile([128, NCH], F32)
            bv_sb = res.tile([128, HID], F32)
            eb_sb = res.tile([128, NHP * 788], F16)
            qt_sb = res.tile([128, NCH * CORE_S], F16)
            kt_sb = res.tile([128, NCH * CORE_S + 64], F16)
            scratch = res.tile([128, 518], F16)   # garbage; warm-up only
            dact = res.tile([128, 2], F16)
            nc.vector.memset(kt_sb[:, NCH * CORE_S:], 0.0)
            vpad = [[vpad_pool.tile([128, HEADS * 65], F16, tag="vp",
                                    name=f"vpad_{b}_{j}") for j in range(2)]
                    for b in range(NB)]

            hs_c = hs_sb.rearrange("p (c x) -> p c x", c=NCH)
            hs8_c = (hs8_sb.rearrange("p (c x) -> p c x", c=NCH)
                     if hs8_sb is not None else None)
            wq_c = wq_sb.rearrange("p (c x) -> p c x", c=NCH)
            wk_c = wk_sb.rearrange("p (c x) -> p c x", c=NCH)
            wq_4 = wq_sb.rearrange("p (c h m) -> p c h m", c=NCH, h=NCH)
            wk_4 = wk_sb.rearrange("p (c h m) -> p c h m", c=NCH, h=NCH)
            wv_c = wv_sb.rearrange("p (c x) -> p c x", c=NCH)
            eb_c = eb_sb.rearrange("p (c x) -> p c x", c=NHP)

            for _ in range(reps):
                # ---- PE warm-up + ACT table-load hoist (zero inputs,
                # outputs never read; runs during the input-DMA window and
                # releases the HAM clock-gate before real matmuls start).
                nc.vector.memset(scratch[:], 0.0)
                nc.scalar.activation(dact[:], scratch[:, :2], AF.Exp)
                dummy_ps = cps_pool.tile([128, 390], F32, tag="c", name="dummy")
                for _k in range(10):
                    nc.tensor.matmul(dummy_ps[:], scratch[:, :128],
                                     scratch[:, 128:518], start=True, stop=True)

                # ---- input DMAs, ordered by first use, rotated over 3 queues ----
                def hs_st(st, c0, c1):
                    return (hs_c[:, c0:c1, st * SW:(st + 1) * SW],
                            hsT_d[st, c0:c1].rearrange("c p x -> p c x"))
                def hs8_st(st, c0, c1):
                    return (hs8_c[:, c0:c1, st * SW:(st + 1) * SW],
                            hs8_d[st, c0:c1].rearrange("c p x -> p c x"))
                fp8 = FP8_Q or FP8_K
                qk_src = hs8_st if fp8 else hs_st
                xfers = [
                    (wq_c[:, 0, :], wq_d[0]),
                    qk_src(0, 0, 3),
                    qk_src(0, 3, 6),
                    (bq_sb[:], bq_d[:, :, 0].rearrange("c p -> p c")),
                    (wk_c[:, 0, :], wk_d[0]),
                    (wq_c[:, 1, :], wq_d[1]),
                    (wk_c[:, 1, :], wk_d[1]),
                ]
                xfers += [
                    (wq_c[:, 2, :], wq_d[2]),
                    (wk_c[:, 2, :], wk_d[2]),
                    (eb_c[:, 0, :], eb_d[0]),
                    (wq_c[:, 3, :], wq_d[3]),
                    (wk_c[:, 3, :], wk_d[3]),
                    (wq_c[:, 4, :], wq_d[4]),
                    (wk_c[:, 4, :], wk_d[4]),
                    (wq_c[:, 5, :], wq_d[5]),
                    (wk_c[:, 5, :], wk_d[5]),
                    qk_src(1, 0, 3),
                    qk_src(1, 3, 6),
                    (wv_c[:, 0:2, :], wv_d[0:2].rearrange("c p x -> p c x")),
                    (wv_c[:, 2:4, :], wv_d[2:4].rearrange("c p x -> p c x")),
                    (wv_c[:, 4:6, :], wv_d[4:6].rearrange("c p x -> p c x")),
                ]
                if fp8:
                    xfers += [hs_st(0, 0, 3), hs_st(0, 3, 6)]
                xfers += [
                    (bv_sb[:], bv_d[:]),
                    (eb_c[:, 1, :], eb_d[1]),
                    (eb_c[:, 2:, :], eb_d[2:].rearrange("c p x -> p c x")),
                    qk_src(2, 0, 6),
                    qk_src(3, 0, 6),
                ]
                if fp8:
                    xfers += [hs_st(1, 0, 3), hs_st(1, 3, 6),
                              hs_st(2, 0, 3), hs_st(2, 3, 6),
                              hs_st(3, 0, 3), hs_st(3, 3, 6)]
                dma_engs = [nc.sync, nc.gpsimd, nc.scalar]
                for xi, (dst, src) in enumerate(xfers):
                    dma_engs[xi % 3].dma_start(dst, src)

                # ---- QK projection group: one (st, c, q|k) psum group ----
                def emit_qk(st, c, which, pool_tag):
                    pool, tag = pool_tag
                    pp = pool.tile([128, 1024] if tag == "s" else [128, SW], F32,
                                   tag=tag, name=f"{which}p_{st}_{c}")
                    use8 = FP8_Q if which == "q" else FP8_K
                    if use8:
                        w_4 = wq_4 if which == "q" else wk_4
                        for t in range(NCH // 2):
                            nc.tensor.matmul(
                                pp[:, :SW],
                                w_4[:, c, 2 * t:2 * t + 2, :],
                                hs8_c[:, 2 * t:2 * t + 2, st * SW:(st + 1) * SW],
                                start=(t == 0), stop=(t == NCH // 2 - 1),
                                perf_mode=DR)
                    else:
                        w_sb = wq_sb if which == "q" else wk_sb
                        for hch in range(NCH):
                            nc.tensor.matmul(
                                pp[:, :SW],
                                w_sb[:, c * HID + hch * 128: c * HID + (hch + 1) * 128],
                                hs_sb[:, hch * CORE_S + st * SW: hch * CORE_S + (st + 1) * SW],
                                start=(hch == 0), stop=(hch == NCH - 1))
                    if which == "q":
                        nc.vector.tensor_scalar_add(
                            qt_sb[:, c * CORE_S + st * SW: c * CORE_S + (st + 1) * SW],
                            pp[:, :SW], bq_sb[:, c:c + 1])
                    else:
                        nc.vector.tensor_copy(
                            kt_sb[:, c * CORE_S + st * SW: c * CORE_S + (st + 1) * SW],
                            pp[:, :SW])

                # ---- V projection for (batch, j-chunk) ----
                def emit_v(b, jci):
                    joff, jlen = JC[jci]
                    vt = vpad[b][jci]
                    ones_ap = vt[:jlen].rearrange("p (h c) -> p h c", h=HEADS)[:, :, 64:65]
                    nc.gpsimd.memset(ones_ap, 1.0)
                    scol = b * S + joff
                    for nt, (noff, nlen) in enumerate([(0, 512), (512, 256)]):
                        vp = wps.tile([128, 512], F32, tag="w",
                                      name=f"vp_{b}_{jci}_{nt}")
                        for c in range(NCH):
                            nc.tensor.matmul(
                                vp[:jlen, :nlen],
                                hs_sb[:, c * CORE_S + scol: c * CORE_S + scol + jlen],
                                wv_sb[:, c * HID + noff: c * HID + noff + nlen],
                                start=(c == 0), stop=(c == NCH - 1))
                        dst = vt[:jlen, nt * 8 * 65:].rearrange(
                            "p (h c) -> p h c", c=65)[:, :nlen // 64, :64]
                        nc.vector.tensor_tensor(
                            out=dst, in0=vp[:jlen, :nlen],
                            in1=bv_sb[:jlen, noff:noff + nlen],
                            op=ALU.add)

                # ---- prologue: QK(st0); V(b0)/V(b1) are the first fill
                # units of pair 0 so they land between attention fronts,
                # right when wv/hs arrive. Two extra warm-keeper dummies
                # bridge any DMA wait so the HAM gate stays open.
                pat = [(wps, "w"), (wps, "w"), (sps, "s")]
                gi = 0
                for c in range(NCH):
                    for which in ("q", "k"):
                        emit_qk(0, c, which, pat[gi % 3])
                        gi += 1
                for _k in range(3):
                    nc.tensor.matmul(dummy_ps[:], scratch[:, :128],
                                     scratch[:, 128:518], start=True, stop=True)

                # ---- attention ----
                def emit_front(b, hp, mul_eng):
                    col = hp * CORE_S + b * S
                    sp = sps.tile([128, 1024], F32, tag="s", name=f"sp_{b}_{hp}")
                    for jci in range(2):
                        for h in range(2):
                            nc.tensor.matmul(
                                sp[:, h * 512 + jci * S: h * 512 + jci * S + S],
                                kt_sb[h * 64:(h + 1) * 64,
                                      col + jci * 128: col + jci * 128 + 128],
                                qt_sb[h * 64:(h + 1) * 64, col: col + S],
                                start=True, stop=True)
                    er = er_pool.tile([128, 788], F16, tag="er", name=f"er_{b}_{hp}")
                    nc.scalar.activation(
                        er.rearrange("p (h x) -> p h x", h=2),
                        sp.rearrange("p (h x) -> p h x", h=2)[:, :, :394], AF.Exp,
                        scale=EXP_SCALE)
                    et = et_pool.tile([128, 788], F16, tag="et", name=f"et_{b}_{hp}")
                    mul_eng.tensor_tensor(
                        out=et[:], in0=er[:],
                        in1=eb_sb[:, hp * 788:(hp + 1) * 788], op=ALU.mult)
                    return [(et, 0), (et, 394)]

                def emit_front_single(b, hp):
                    # tail variant: per-head PSUM tiles from the (now idle)
                    # wp pool so the exp->scores chain double-buffers even
                    # though the sp pool has a single slot; eb multiplies
                    # alternate DVE/GpSimd to spread the un-overlapped tail.
                    col = hp * CORE_S + b * S
                    ets = []
                    for h in range(2):
                        sph = wps.tile([128, SW], F32, tag="w",
                                       name=f"sps_{b}_{hp}_{h}")
                        for jci in range(2):
                            nc.tensor.matmul(
                                sph[:, jci * S: jci * S + S],
                                kt_sb[h * 64:(h + 1) * 64,
                                      col + jci * 128: col + jci * 128 + 128],
                                qt_sb[h * 64:(h + 1) * 64, col: col + S],
                                start=True, stop=True)
                        er = er_pool.tile([128, 394], F16, tag="er",
                                          name=f"ers_{b}_{hp}_{h}")
                        nc.scalar.activation(er[:], sph[:, :SW], AF.Exp,
                                             scale=EXP_SCALE)
                        et = et_pool.tile([128, 394], F16, tag="et",
                                          name=f"ets_{b}_{hp}_{h}")
                        eng = nc.gpsimd if (hp + h) % 2 else nc.vector
                        eng.tensor_tensor(
                            out=et[:], in0=er[:],
                            in1=eb_sb[:, hp * 788 + h * 394: hp * 788 + (h + 1) * 394],
                            op=ALU.mult)
                        ets.append((et, 0))
                    return ets

                def emit_ctx(b, half, hpl, ets, cps):
                    for ici, (ioff, ilen) in enumerate(JC):
                        for h in range(2):
                            et, base = ets[h]
                            for jci, (joff, jlen) in enumerate(JC):
                                nc.tensor.matmul(
                                    cps[ici][:ilen, hpl * 130 + h * 65:
                                             hpl * 130 + (h + 1) * 65],
                                    et[:jlen, base + jci * S + ioff:
                                       base + jci * S + ioff + ilen],
                                    vpad[b][jci][:jlen,
                                                 ((half * 3 + hpl) * 2 + h) * 65:
                                                 ((half * 3 + hpl) * 2 + h + 1) * 65],
                                    start=(jci == 0), stop=(jci == 1))

                def attn(b, fill, single=False, fronts_first=False):
                    ot = [ot_pool.tile([128, HID], F32, tag="ot",
                                       name=f"ot_{b}_{i}") for i in range(2)]
                    for half in range(2):
                        cps = [cps_pool.tile([128, 390], F32, tag="c",
                                             name=f"cp_{b}_{half}_{i}")
                               for i in range(2)]
                        if fronts_first:
                            # first pair: V hasn't landed yet, so emit all
                            # fronts (which only need qt/kt/eb) before any
                            # ctx matmul touches vpad.
                            etss = []
                            for hpl in range(3):
                                etss.append(emit_front(b, half * 3 + hpl,
                                                       nc.vector))
                                fill()
                            fill()
                            for hpl in range(3):
                                emit_ctx(b, half, hpl, etss[hpl], cps)
                        else:
                            prev = None
                            for hpl in range(3):
                                hp = half * 3 + hpl
                                if single:
                                    ets = emit_front_single(b, hp)
                                else:
                                    ets = emit_front(b, hp, nc.vector)
                                fill()
                                if prev is not None:
                                    emit_ctx(b, half, prev[0], prev[1], cps)
                                prev = (hpl, ets)
                            emit_ctx(b, half, prev[0], prev[1], cps)
                            fill()
                        for ici, (ioff, ilen) in enumerate(JC):
                            r = rt_pool.tile([128, 6], F32, tag="rt",
                                             name=f"r_{b}_{half}_{ici}")
                            sums = cps[ici][:ilen].rearrange(
                                "p (g c) -> p g c", c=65)[:, :, 64:65]
                            nc.vector.reciprocal(r[:ilen], sums)
                            nc.vector.tensor_tensor(
                                out=ot[ici][:ilen, half * 384:(half + 1) * 384]
                                    .rearrange("p (g c) -> p g c", c=64),
                                in0=cps[ici][:ilen].rearrange(
                                    "p (g c) -> p g c", c=65)[:, :, :64],
                                in1=r[:ilen].broadcast_to([ilen, 6, 64]),
                                op=ALU.mult)
                            # per-(half, ici) output DMA: half 0's write
                            # overlaps half 1's compute (matters for the
                            # last batch's un-overlapped tail)
                            nc.sync.dma_start(
                                y_d[b, ioff:ioff + ilen,
                                    half * 384:(half + 1) * 384],
                                ot[ici][:ilen, half * 384:(half + 1) * 384])

                for p in range(NST):
                    units = []
                    qk_units = ([(p + 1, c, w) for c in range(NCH)
                                 for w in ("q", "k")] if p + 1 < NST else [])
                    if p == 0:
                        # qk(1) c0 first (keeps PE fed while wv streams in),
                        # then this pair's own V right before its ctx needs it
                        units += [("qk", qk_units[0]), ("qk", qk_units[1]),
                                  ("v", (0, 0)), ("v", (0, 1)),
                                  ("v", (1, 0)), ("v", (1, 1))]
                        qk_units = qk_units[2:]
                    if p + 1 < NST:
                        v_units = [(2 * p + 2, 0), (2 * p + 2, 1),
                                   (2 * p + 3, 0), (2 * p + 3, 1)]
                        # qk,qk,qk,v interleave
                        qi = vi = 0
                        while qi < len(qk_units) or vi < len(v_units):
                            for _ in range(3):
                                if qi < len(qk_units):
                                    units.append(("qk", qk_units[qi])); qi += 1
                            if vi < len(v_units):
                                units.append(("v", v_units[vi])); vi += 1
                    units.reverse()

                    def fill():
                        if units:
                            kind, u = units.pop()
                            if kind == "qk":
                                st, c, w = u
                                emit_qk(st, c, w, (wps, "w"))
                            else:
                                emit_v(*u)

                    last = p == NST - 1
                    attn(2 * p, fill, single=last, fronts_first=(p == 0))
                    attn(2 * p + 1, fill, single=last, fronts_first=(p == 0))
                    while units:
                        fill()

    nc.compile()
    return nc


_NC_CACHE = {}


def _get_nc(reps=1):
    if reps not in _NC_CACHE:
        _NC_CACHE[reps] = build_nc(reps)
    return _NC_CACHE[reps]


def prep_inputs(hidden_states, Wq, bq, Wk, Wv, bv, bias_table):
    hidden_states = np.asarray(hidden_states, np.float32)
    Wq = np.asarray(Wq, np.float32)
    bq = np.asarray(bq, np.float32)
    Wk = np.asarray(Wk, np.float32)
    Wv = np.asarray(Wv, np.float32)
    bv = np.asarray(bv, np.float32)
    bias_table = np.asarray(bias_table, np.float32)

    def cmajor(wT):
        # [h_in, d_out] -> [c, p, hch*128+col] so one DMA covers one d_out chunk
        return np.ascontiguousarray(
            wT.reshape(NCH, 128, NCH, 128).transpose(2, 1, 0, 3).reshape(NCH, 128, HID))
    wqT = cmajor((Wq * QSCALE).T).astype(NPF8 if FP8_Q else np.float16)
    wkT = cmajor((Wk * KSCALE).T).astype(NPF8 if FP8_K else np.float16)
    wvT = np.ascontiguousarray(Wv.T).reshape(NCH, 128, HID).astype(np.float16)
    bqc = (bq * QSCALE).astype(np.float32).reshape(NCH, 128, 1)
    bvb = np.ascontiguousarray(np.broadcast_to(bv, (128, HID))).astype(np.float32)

    idx = _relative_position_index(14, 14)
    bias_full = bias_table[idx]              # [S, S, HEADS] (i, j, h)
    biasT = bias_full.transpose(2, 1, 0)     # [h, j, i]
    expb = np.zeros((NHP, 128, 788), np.float32)
    for c in range(NHP):
        for h in range(2):
            for jci, (joff, jlen) in enumerate(JC):
                expb[c, :jlen, h * 394 + jci * S: h * 394 + jci * S + S] = \
                    np.exp(biasT[2 * c + h, joff:joff + jlen, :])
    expb = expb.astype(np.float16)

    shared = {"wqT": wqT, "wkT": wkT, "wvT": wvT, "bqc": bqc, "bvb": bvb,
              "expb": expb}
    in_maps = []
    for core in range(N_CORES):
        hs_c = hidden_states[core * NB:(core + 1) * NB]      # [NB, S, HID]
        hsT = np.ascontiguousarray(hs_c.transpose(2, 0, 1).reshape(HID, CORE_S))
        hsT4 = np.ascontiguousarray(
            hsT.reshape(NCH, 128, NST, SW).transpose(2, 0, 1, 3))
        im = {"hsT": hsT4.astype(np.float16), **shared}
        if FP8_Q or FP8_K:
            im["hs8T"] = hsT4.astype(NPF8)
        in_maps.append(im)
    return in_maps


def run(in_maps, reps=1, **kw):
    nc = _get_nc(reps)
    res = run_bass_kernel_spmd(nc, in_maps, core_ids=list(range(N_CORES)), **kw)
    out = np.concatenate([res.results[c]["y"] for c in range(N_CORES)], axis=0)
    return out, res


def kernel(hidden_states, Wq, bq, Wk, Wv, bv, bias_table,
           resolution_h=224, resolution_w=224):
    assert int(resolution_h) == 224 and int(resolution_w) == 224, \
        "kernel compiled for 224x224 (window 14x14, S=197)"
    hidden_states = np.asarray(hidden_states)
    assert hidden_states.shape == (B, S, HID), hidden_states.shape
    in_maps = prep_inputs(hidden_states, Wq, bq, Wk, Wv, bv, bias_table)
    return run(in_maps, reps=1)[0]
